# revision 1
# baseline (speedup 1.0000x reference)
"""Trainium2 Bass kernel for nn_Diagomal_DWConv (diagonal depthwise conv).

Math (derived from the reference):
  View x as rows X[r, w], r in [0, R), R = B*C*H, W columns.
  out[r, w] = bias[c(r)] + sum_i weight[c(r), 0, i] * X[(r + 2 - i) mod R, w + i - 2]
  with zero padding in w only, c(r) = (r // H) mod C.

Strategy:
  - One batch (16384 rows) per NeuronCore; host supplies per-core rows with a
    2-row halo on each side and zero-padded columns (row stride 260).
  - Each of the 128 SBUF partitions holds a chunk of 128 consecutive rows
    (+4 halo rows) at row-stride 260 in the free dimension, so every tap of
    the 5-tap diagonal conv is a pure free-dimension offset.
  - The tap accumulation runs on the TensorEngine as 5 PSUM-accumulating
    matmuls whose stationary operands are diagonal matrices
    diag(weight[c(p), i]) in float32r (1 cycle/row; ~2.5e-4 scale-relative
    rounding). Diagonals are built on-chip from identity x per-partition
    weight. Dep-free warm-up matmuls release the PE HAM clock gate during
    the DMA head.
  - ScalarEngine drains PSUM -> SBUF adding the per-partition bias.
  - Input chunks ride the Sync (SP) HWDGE ring with a progressive size
    schedule (small first so compute starts early); output pieces ride the
    GpSimd SWDGE ring so input and output transfers interleave without the
    input stream starving.
"""

import numpy as np

import concourse.bacc as bacc
import concourse.tile as tile
import concourse.mybir as mybir
from concourse.bass_utils import run_bass_kernel_spmd

F32 = mybir.dt.float32
F32R = mybir.dt.float32r

B, C, H, W = 8, 64, 256, 256
KS, PAD = 5, 2
R = B * C * H          # 131072 rows total
NCORES = 8
RC = R // NCORES       # 16384 rows per core (exactly one batch)
WP = W + 2 * PAD       # 260 padded row stride in SBUF
NP = 128               # partitions
G = RC // NP           # 128 output rows per partition chunk
NS = (G * W) // 512    # 64 psum slices (512 f32 each = 2 rows)
CHUNK_ROWS = [6, 6, 8, 10, 14, 18, 22, 26, 22]  # slab load chunks (sum = G+4)
NPIECE = 16            # output drained in pieces
SPP = NS // NPIECE     # slices per out piece
FP_ = SPP * 512        # out piece free elems per partition
NWARM = 64             # PE warm-up matmuls

_CACHE = {}


def _build_nc():
    nc = bacc.Bacc("TRN2", num_devices=NCORES)
    xk = nc.dram_tensor("xk", [(RC + 4) * WP], F32R, kind="ExternalInput")
    idm = nc.dram_tensor("idm", [NP, NP], F32R, kind="ExternalInput")
    wp_ = nc.dram_tensor("wp", [NP, KS], F32, kind="ExternalInput")
    bs = nc.dram_tensor("bs", [NP, 1], F32, kind="ExternalInput")
    yk = nc.dram_tensor("yk", [RC * W], F32, kind="ExternalOutput")

    with tile.TileContext(nc) as tc:
        with (
            tc.tile_pool(name="const", bufs=1) as cpool,
            tc.tile_pool(name="inp", bufs=1) as ipool,
            tc.tile_pool(name="outp", bufs=4) as opool,
            tc.tile_pool(name="ps", bufs=8, space="PSUM") as pspool,
            tc.tile_pool(name="warm", bufs=1) as wpool,
        ):
            # PE warm-up: dep-free tiny matmuls run during the DMA head so
            # the HAM clock gate is released before the real stream starts.
            wt_ = wpool.tile([NP, 256], F32R)
            nc.vector.memset(wt_[:].bitcast(F32), 0.0)
            wps = pspool.tile([NP, 256], F32, tag="ps")
            for _ in range(NWARM):
                nc.tensor.matmul(
                    wps[0:64, 0:256], wt_[:, 0:64], wt_[:, 0:256], start=True, stop=True
                )

            # constants: identity + per-partition weights -> 5 diag stationaries
            idt = cpool.tile([NP, NP], F32R)
            nc.sync.dma_start(idt[:], idm.ap())
            wpt = cpool.tile([NP, KS], F32)
            nc.sync.dma_start(wpt[:], wp_.ap())
            bst = cpool.tile([NP, 1], F32)
            nc.sync.dma_start(bst[:], bs.ap())
            dgt = cpool.tile([NP, KS * NP], F32R)
            for i in range(KS):
                nc.vector.tensor_scalar(
                    dgt[:, i * NP : (i + 1) * NP],
                    idt[:],
                    wpt[:, i : i + 1],
                    None,
                    mybir.AluOpType.mult,
                )

            # input slab: partition p holds padded rows [p*G, p*G + G + 4)
            # at row stride WP (host supplies zero-padded columns).
            it = ipool.tile([NP, (G + 4) * WP], F32R)
            it3 = it.rearrange("p (r c) -> p r c", c=WP)  # [128, G+4, 260]
            ro = 0
            for cr in CHUNK_ROWS:
                src = xk.ap().copy()
                src.ap = mybir.VecI64Pair([[G * WP, NP], [1, cr * WP]])
                src.offset = ro * WP
                nc.sync.dma_start(it[:, ro * WP : (ro + cr) * WP], src)
                ro += cr

            # output viewed as [piece, partition, free]
            yv = yk.ap().rearrange("(p q f) -> q p f", p=NP, q=NPIECE)

            ot = None
            for s in range(NS):
                qi, sq = divmod(s, SPP)
                if sq == 0:
                    ot = opool.tile([NP, FP_], F32)
                ps = pspool.tile([NP, 512], F32)
                for i in range(KS):
                    mov = it3[:, 2 * s + 4 - i : 2 * s + 6 - i, i : i + W]
                    nc.tensor.matmul(
                        ps[:],
                        dgt[:, i * NP : (i + 1) * NP],
                        mov,
                        start=(i == 0),
                        stop=(i == KS - 1),
                    )
                nc.scalar.activation(
                    ot[:, sq * 512 : (sq + 1) * 512],
                    ps[:],
                    mybir.ActivationFunctionType.Identity,
                    bias=bst[:, 0:1],
                    scale=1.0,
                )
                if sq == SPP - 1:
                    nc.gpsimd.dma_start(yv[qi], ot[:])

    nc.compile()
    return nc


def _host_prep(x, weight, bias):
    """Returns per-core in_maps (row slab with halo + diag stationaries + bias)."""
    xr = np.ascontiguousarray(x, dtype=np.float32).reshape(R, W)
    pall = np.zeros((R + 4, WP), dtype=np.float32)
    pall[2 : R + 2, PAD : PAD + W] = xr
    pall[0:2, PAD : PAD + W] = xr[R - 2 : R]
    pall[R + 2 : R + 4, PAD : PAD + W] = xr[0:2]

    chan = (np.arange(NP) * G) // H  # channel of partition p's chunk
    wgt = np.ascontiguousarray(weight, dtype=np.float32).reshape(C, KS)
    wpp = np.ascontiguousarray(wgt[chan], dtype=np.float32)  # [NP, KS]
    idv = np.eye(NP, dtype=np.float32)
    bs_arr = np.ascontiguousarray(bias, dtype=np.float32)[chan].reshape(NP, 1)

    in_maps = []
    for k in range(NCORES):
        pk = pall[k * RC : k * RC + RC + 4].reshape(-1)
        in_maps.append({"xk": pk, "idm": idv, "wp": wpp, "bs": bs_arr})
    return in_maps


def kernel(x, weight, bias):
    x = np.asarray(x)
    weight = np.asarray(weight)
    bias = np.asarray(bias)
    if "nc" not in _CACHE:
        _CACHE["nc"] = _build_nc()
    nc = _CACHE["nc"]
    in_maps = _host_prep(x, weight, bias)
    res = run_bass_kernel_spmd(nc, in_maps, list(range(NCORES)))
    out = np.stack([res.results[k]["yk"] for k in range(NCORES)])
    return out.reshape(B, C, H, W).astype(np.float32)

